# revision 89
# baseline (speedup 1.0000x reference)
"""Trainium2 Bass kernel for MllamaTextSdpaAttention (GQA + RoPE + causal SDPA).

Strategy: tensor-parallel over heads across 8 NeuronCores. Core c owns
q-heads [4c, 4c+4) and kv-head c (kv groups intact). Each core computes
hidden @ Wq/Wk/Wv slices, RoPE, causal attention for its heads, and its
row-slice of the Wo matmul, yielding a partial [T, DIM] output (bf16).
The host sums the 8 partials in f32.

Key techniques:
- All four projections (Q/K/V/O) run on the PE in fp8e4m3 DoubleRow mode
  (2 k-tiles of contraction per instruction at 0.5 cycles/column = 4x the
  bf16 FLOP rate). Accuracy is preserved with a 3-term residual split:
  each operand X is split (host-side for inputs/weights, on-device for
  ao) into Xh = fp8(X), Xl = fp8(X - Xh), and W@X ~= Wh@Xh + Wl@Xh +
  Wh@Xl. Net cost: 0.75x the bf16 column count. Weights are pre-scaled
  (x32 Wq/Wk/Wo, x16 Wv) into e4m3's normal range; descales fold into
  the RoPE tables and the host-side gather.
- All inputs are pre-swizzled on the host into exact SBUF layouts so
  every DMA moves >=512-byte contiguous runs (full 360 GB/s; under 512B
  the DMA engines run at half rate).
- Attention stays bf16 on the PE (scores + P@V only): transposed scores
  (scT = K_rot^T.T @ Q_rot^T), exp on Act feeds P@V directly. The
  softmax rowsums are computed OFF the PE: et tiles are summed
  elementwise on DVE (bf16, 2x/4x modes), then one GpSimd
  partition_all_reduce broadcasts the rowsum to all partitions; the
  reciprocal+normalize epilogue is deferred one group so the PE never
  waits on it. RoPE as a half-rotation with host-permuted weight
  columns. Causality at 128-block granularity.
- Schedule: per chunk, K and V run first (term-staged against DMA
  arrival for chunk 0), then Q heads emitted in sub-batches with the
  previous head's attention-group units interleaved into the stream
  (group work is Act-heavy, projections are PE-heavy). The O projection
  for chunk c-1 interleaves with chunk c's last two groups and paces
  the next chunk's hs DMAs, spreading output DMA across the kernel.
  Only chunk 3's O-proj trails the last attention group, with its
  epilogue chain hidden under chunk 2's O-proj.
"""

import numpy as np
import ml_dtypes

import concourse.bacc as bacc
import concourse.bass as bass
import concourse.bass_isa as bass_isa
import concourse.mybir as mybir
from concourse.tile import TileContext
from concourse import bass_utils

BF16 = mybir.dt.bfloat16
F32 = mybir.dt.float32
F8 = mybir.dt.float8e4
E4M3 = ml_dtypes.float8_e4m3

B, S, DIM = 2, 1024, 4096
T = B * S                     # 2048 tokens, batch-major
N_HEADS, N_KV = 32, 8
HD = 128                      # head dim == partition count
N_CORES = 8
HL = N_HEADS // N_CORES       # 4 local q-heads per core
KT = DIM // 128               # 32 feature tiles
KP = KT // 2                  # 16 k-tile PAIRS (DoubleRow)
CH = 512                      # projection token-chunk
NCHUNK = T // CH
QB = 512                      # attention q-block width
TT = T // 128                 # 16 token tiles global
SCALE = 1.0 / float(np.sqrt(HD))
S_Q = 32.0                    # weight pre-scales for fp8 range
S_K = 32.0
S_V = 16.0
S_O = 32.0
DR = mybir.MatmulPerfMode.DoubleRow

_CACHE: dict = {}


def _build():
    nc = bacc.Bacc("TRN2", target_bir_lowering=False, debug=False,
                   enable_asserts=False, dynamic_dma_scratch_size=2048)

    # all tensors pre-swizzled host-side into SBUF layout (partition-major)
    hsh_d = nc.dram_tensor("hsh", [128, NCHUNK, KT, CH], F8, kind="ExternalInput")
    hsl_d = nc.dram_tensor("hsl", [128, NCHUNK, KT, CH], F8, kind="ExternalInput")
    wqh_d = nc.dram_tensor("wqh", [128, HL, KT, HD], F8, kind="ExternalInput")
    wql_d = nc.dram_tensor("wql", [128, HL, KT, HD], F8, kind="ExternalInput")
    wkh_d = nc.dram_tensor("wkh", [128, KT, HD], F8, kind="ExternalInput")
    wkl_d = nc.dram_tensor("wkl", [128, KT, HD], F8, kind="ExternalInput")
    wvh_d = nc.dram_tensor("wvh", [128, KT, HD], F8, kind="ExternalInput")
    wvl_d = nc.dram_tensor("wvl", [128, KT, HD], F8, kind="ExternalInput")
    woh_d = nc.dram_tensor("woh", [128, HL, DIM], F8, kind="ExternalInput")
    wol_d = nc.dram_tensor("wol", [128, HL, DIM], F8, kind="ExternalInput")
    cos_q = nc.dram_tensor("cos_q", [HD, T], BF16, kind="ExternalInput")
    sin_q = nc.dram_tensor("sin_q", [HD, T], BF16, kind="ExternalInput")
    cos_k = nc.dram_tensor("cos_k", [HD, T], BF16, kind="ExternalInput")
    sin_k = nc.dram_tensor("sin_k", [HD, T], BF16, kind="ExternalInput")
    maskT = nc.dram_tensor("maskT", [128, 128], F32, kind="ExternalInput")
    out = nc.dram_tensor("out", [T, DIM], BF16, kind="ExternalOutput")

    Exp = mybir.ActivationFunctionType.Exp

    with TileContext(nc) as tc:
        with tc.tile_pool(name="consts", bufs=1) as cpool, \
             tc.tile_pool(name="hs", bufs=2) as hpool, \
             tc.tile_pool(name="rope_tmp", bufs=1) as rpool, \
             tc.tile_pool(name="work_ps", bufs=6, space=bass.MemorySpace.PSUM) as wpool, \
             tc.tile_pool(name="ot_ps", bufs=2, space=bass.MemorySpace.PSUM) as otpool, \
             tc.tile_pool(name="et", bufs=4) as epool, \
             tc.tile_pool(name="esum", bufs=1) as espool, \
             tc.tile_pool(name="tao", bufs=1) as taopool, \
             tc.tile_pool(name="out_sb", bufs=4) as xsbpool:

            wqh_t = [cpool.tile([128, KT, HD], F8, tag=f"wqh{m}", name=f"wqh{m}")
                     for m in range(HL)]
            wql_t = [cpool.tile([128, KT, HD], F8, tag=f"wql{m}", name=f"wql{m}")
                     for m in range(HL)]
            wkh_t = cpool.tile([128, KT, HD], F8, tag="wkh")
            wkl_t = cpool.tile([128, KT, HD], F8, tag="wkl")
            wvh_t = cpool.tile([128, KT, HD], F8, tag="wvh")
            wvl_t = cpool.tile([128, KT, HD], F8, tag="wvl")
            woh_sb = cpool.tile([128, HL, DIM], F8, tag="woh")
            wol_sb = cpool.tile([128, HL, DIM], F8, tag="wol")
            cq_sb = cpool.tile([128, T], BF16, tag="cq")
            sq_sb = cpool.tile([128, T], BF16, tag="sq")
            ck_sb = cpool.tile([128, T], BF16, tag="ck")
            sk_sb = cpool.tile([128, T], BF16, tag="sk")
            maskT_sb = cpool.tile([128, 128], F32, tag="maskT")
            qt_rot = cpool.tile([128, HL, T], BF16, tag="qt")
            kt_rot = cpool.tile([128, T], BF16, tag="kt")
            v_sb = cpool.tile([128, TT, HD], BF16, tag="v")
            aoh = cpool.tile([128, HL, T], F8, tag="aoh")
            aol = cpool.tile([128, HL, T], F8, tag="aol")

            # startup-critical DMA first: K-projection weights
            nc.sync.dma_start(wkh_t[:, 0:8, :], wkh_d.ap()[:, 0:8, :])
            nc.sync.dma_start(wkh_t[:, 8:KT, :], wkh_d.ap()[:, 8:KT, :])
            nc.sync.dma_start(wkl_t, wkl_d.ap())

            def emit_hs_dmas(c, lo=True):
                hsh_sb = hpool.tile([128, KT, CH], F8, tag="hsh", name="hsh_sb")
                hsl_sb = hpool.tile([128, KT, CH], F8, tag="hsl", name="hsl_sb")
                for g in range(4):
                    nc.sync.dma_start(hsh_sb[:, g * 8:(g + 1) * 8, :],
                                      hsh_d.ap()[:, c, g * 8:(g + 1) * 8, :])
                if lo:
                    for g in range(4):
                        nc.sync.dma_start(hsl_sb[:, g * 8:(g + 1) * 8, :],
                                          hsl_d.ap()[:, c, g * 8:(g + 1) * 8, :])
                return hsh_sb, hsl_sb

            def late_consts(hsl_sb):
                # strictly ordered by first use under the term-staged chunk-0
                # emission: V terms, then K-hl/V-lh (hsl), then Q0, ropes, Q1+
                nc.sync.dma_start(wvh_t, wvh_d.ap())
                nc.sync.dma_start(wvl_t, wvl_d.ap())
                nc.sync.dma_start(wqh_t[0], wqh_d.ap()[:, 0])
                nc.sync.dma_start(wql_t[0], wql_d.ap()[:, 0])
                # cos/sin tables: chunk 0 only needs its own 512-col slices
                # now; the rest ride along with later chunks' hs feeds
                nc.sync.dma_start(cq_sb[:, 0:CH], cos_q.ap()[:, 0:CH])
                nc.sync.dma_start(sq_sb[:, 0:CH], sin_q.ap()[:, 0:CH])
                nc.sync.dma_start(maskT_sb, maskT.ap())
                for g in range(4):
                    nc.sync.dma_start(hsl_sb[:, g * 8:(g + 1) * 8, :],
                                      hsl_d.ap()[:, 0, g * 8:(g + 1) * 8, :])
                nc.sync.dma_start(ck_sb[:, 0:CH], cos_k.ap()[:, 0:CH])
                nc.sync.dma_start(sk_sb[:, 0:CH], sin_k.ap()[:, 0:CH])
                nc.sync.dma_start(wqh_t[1], wqh_d.ap()[:, 1])
                nc.sync.dma_start(wql_t[1], wql_d.ap()[:, 1])
                for m in range(2, HL):
                    nc.sync.dma_start(wqh_t[m], wqh_d.ap()[:, m])
                    nc.sync.dma_start(wql_t[m], wql_d.ap()[:, m])

            def rope(ps, out_ap, cos_ap, sin_ap):
                """out = ps*cos + halfswap(ps)*sin  (signs baked into sin)."""
                t1 = rpool.tile([128, CH], F32, tag="r1", name="t1")
                t2 = rpool.tile([128, CH], F32, tag="r2", name="t2")
                nc.vector.tensor_mul(t1, ps, cos_ap)
                nc.vector.tensor_mul(t2[0:64, :], ps[64:128, :], sin_ap[0:64, :])
                nc.vector.tensor_mul(t2[64:128, :], ps[0:64, :], sin_ap[64:128, :])
                nc.vector.tensor_add(out_ap, t1, t2)

            def mm3(ps, st_h, st_l, mv_h, mv_l):
                """3-term fp8 DoubleRow accumulation over all KT k-tiles.
                The two mv_h terms interleave per k-pair so each arriving
                hs piece carries twice the compute (K/V are DMA-paced at
                chunk starts)."""
                for kp in range(KP):
                    nc.tensor.matmul(ps, st_h(kp), mv_h(kp),
                                     start=(kp == 0), stop=False, perf_mode=DR)
                for kp in range(KP):
                    nc.tensor.matmul(ps, st_l(kp), mv_h(kp),
                                     start=False, stop=False, perf_mode=DR)
                for kp in range(KP):
                    nc.tensor.matmul(ps, st_h(kp), mv_l(kp),
                                     start=False, stop=(kp == KP - 1),
                                     perf_mode=DR)

            # --- attention group machinery (transposed-scores scheme) ---
            pending = [None]

            def epilogue(st):
                rs, ot, h, q0 = st
                with nc.allow_low_precision("softmax rowsum recip in bf16"):
                    nc.vector.reciprocal(rs, rs)
                t = taopool.tile([128, QB], F32, tag="tao", name="tao")
                nc.vector.tensor_mul(t, ot, rs)
                nc.scalar.copy(aoh[:, h, q0:q0 + QB], t)
                nc.vector.tensor_sub(aol[:, h, q0:q0 + QB], t,
                                     aoh[:, h, q0:q0 + QB])

            def group_units(b, h, qb):
                """Generator: one yield per consumed score k-tile, so group
                work (Act-heavy exp) can be interleaved into PE-heavy Q/O
                projection streams."""
                q0 = b * S + qb * QB
                n_kt = (qb + 1) * (QB // 128)
                # esum accumulates sum_kt et_kt elementwise on DVE (bf16, 2x
                # mode); the final GpSimd partition_all_reduce turns it into
                # softmax rowsums broadcast across partitions. Keeps the
                # rowsum off the PE; bf16 accumulation costs ~0.5% on rs,
                # well inside the error budget.
                esum = espool.tile([128, QB], BF16, tag="esum", name="esum")
                ot = otpool.tile([128, QB], F32, tag="ot", name="ot")
                ets = [None] * n_kt

                def emit_sc(kt):
                    c0 = max(0, kt - qb * (QB // 128)) * 128
                    sc = wpool.tile([128, QB], F32, tag="work", name="sc")
                    nc.tensor.matmul(
                        sc[:, c0:],
                        kt_rot[:, b * S + kt * 128:b * S + (kt + 1) * 128],
                        qt_rot[:, h, q0 + c0:q0 + QB],
                        start=True, stop=True)
                    jd = kt - qb * (QB // 128)
                    if 0 <= jd < QB // 128:
                        nc.vector.tensor_add(sc[:, jd * 128:(jd + 1) * 128],
                                             sc[:, jd * 128:(jd + 1) * 128],
                                             maskT_sb)
                    et = epool.tile([128, QB], BF16, tag="et", name="et")
                    nc.scalar.activation(et[:, c0:], sc[:, c0:], Exp,
                                         bias=0.0, scale=1.0)
                    ets[kt] = (et, c0)

                def consume(kt):
                    et, c0 = ets[kt]
                    if kt == 0:
                        nc.vector.tensor_copy(esum, et)
                    else:
                        nc.vector.tensor_add(esum[:, c0:], esum[:, c0:],
                                             et[:, c0:])
                    nc.tensor.matmul(ot[:, c0:], v_sb[:, b * (S // 128) + kt, :],
                                     et[:, c0:], start=(kt == 0),
                                     stop=(kt == n_kt - 1))
                    ets[kt] = None
                    if kt == 0 and pending[0] is not None:
                        epilogue(pending[0])
                        pending[0] = None

                for kt in range(n_kt):
                    emit_sc(kt)
                    if kt >= 2:
                        consume(kt - 2)
                        yield
                for kt in range(max(0, n_kt - 2), n_kt):
                    consume(kt)
                    yield
                nc.gpsimd.partition_all_reduce(esum, esum, 128,
                                               bass_isa.ReduceOp.add)
                pending[0] = (esum, ot, h, q0)

            def oproj_units(c, direct_out=False):
                """Generator: one yield per O-projection psum tile (fp8
                DoubleRow) for chunk c's 4 token tiles. Two 512-col psum
                tiles pair into one [128,1024] osb buffer and ONE output
                DMA, halving the HWDGE issue pressure (625ns per DMA is
                otherwise at parity with the PE's tile rate)."""
                for tt in range(c * 4, c * 4 + 4):
                    ts = tt * 128
                    for n0 in range(0, DIM, 1024):
                        osb = xsbpool.tile([128, 1024], BF16, tag="osb",
                                           name="osb")
                        on_act = (tt * 4 + n0 // 1024) % 2 == 0
                        for half in range(2):
                            nh = n0 + half * 512
                            ps = wpool.tile([128, 512], F32, tag="work",
                                            name="ps_o")
                            # hp-outer order: the head-pair (0,1) terms run
                            # before any (2,3) term, covering the last
                            # group's epilogue-chain latency with real work
                            for hp in range(2):
                                h2 = 2 * hp
                                nc.tensor.matmul(
                                    ps, aoh[:, h2:h2 + 2, ts:ts + 128],
                                    woh_sb[:, h2:h2 + 2, nh:nh + 512],
                                    start=(hp == 0), stop=False, perf_mode=DR)
                                nc.tensor.matmul(
                                    ps, aoh[:, h2:h2 + 2, ts:ts + 128],
                                    wol_sb[:, h2:h2 + 2, nh:nh + 512],
                                    start=False, stop=False, perf_mode=DR)
                                nc.tensor.matmul(
                                    ps, aol[:, h2:h2 + 2, ts:ts + 128],
                                    woh_sb[:, h2:h2 + 2, nh:nh + 512],
                                    start=False, stop=(hp == 1), perf_mode=DR)
                            dst = osb[:, half * 512:(half + 1) * 512]
                            if on_act:
                                nc.scalar.copy(dst, ps)
                            else:
                                nc.vector.tensor_copy(dst, ps)
                            if half == 0:
                                yield
                        nc.sync.dma_start(
                            out.ap()[ts:ts + 128, n0:n0 + 1024], osb)
                        yield

            def drain(gen, n=10 ** 9):
                """Pull up to n units; True if the generator is exhausted."""
                for _ in range(n):
                    if next(gen, _SENTINEL) is _SENTINEL:
                        return True
                return False

            _SENTINEL = object()

            def hs_dma_closures(c):
                """Allocate next chunk's hs tiles; return deferred DMA
                emitters so the transfers can be paced into the O-proj
                stream (fair-sharing the DMA engines with osb writes)."""
                hsh_sb = hpool.tile([128, KT, CH], F8, tag="hsh", name="hsh_sb")
                hsl_sb = hpool.tile([128, KT, CH], F8, tag="hsl", name="hsl_sb")

                def mk(dst, src, g):
                    return lambda: nc.sync.dma_start(
                        dst[:, g * 8:(g + 1) * 8, :],
                        src[:, c, g * 8:(g + 1) * 8, :])

                def mk_cs(dst, src):
                    return lambda: nc.sync.dma_start(
                        dst[:, c * CH:(c + 1) * CH],
                        src[:, c * CH:(c + 1) * CH])

                fs = [mk(hsh_sb, hsh_d.ap(), g) for g in range(4)]
                fs += [mk_cs(ck_sb, cos_k.ap()), mk_cs(sk_sb, sin_k.ap())]
                fs += [mk(hsl_sb, hsl_d.ap(), g) for g in range(4)]
                fs += [mk_cs(cq_sb, cos_q.ap()), mk_cs(sq_sb, sin_q.ap())]
                return (hsh_sb, hsl_sb), fs

            # --- main schedule ---
            hs_cur = emit_hs_dmas(0, lo=False)
            for c in range(NCHUNK):
                hsh_sb, hsl_sb = hs_cur
                t0 = c * CH
                b, qb = c // 2, c % 2
                def st(w):
                    return lambda kp: w[:, 2 * kp:2 * kp + 2, :]

                def mv(x):
                    return lambda kp: x[:, 2 * kp:2 * kp + 2, :]

                def mm(ps, s, v, start=False, stop=False):
                    nc.tensor.matmul(ps, s, v, start=start, stop=stop,
                                     perf_mode=DR)

                if True:
                    # Term-staged K/V/Q0 for every chunk, ordered to match
                    # DMA arrival: K-hh/lh and V-hh/hl need only hsh; the
                    # hsl-dependent third terms run after, so the PE never
                    # waits on hsl at a chunk boundary.
                    kh, kl = st(wkh_t), st(wkl_t)
                    vh, vl = st(wvh_t), st(wvl_t)
                    qh, ql = st(wqh_t[0]), st(wql_t[0])
                    mh, ml = mv(hsh_sb), mv(hsl_sb)
                    psK = wpool.tile([128, CH], F32, tag="work", name="ps_k")
                    for kp in range(KP):
                        mm(psK, kh(kp), mh(kp), start=(kp == 0))
                    if c == 0:
                        late_consts(hsl_sb)
                    for kp in range(KP):
                        mm(psK, kl(kp), mh(kp))
                    psV = []
                    for vi in range(CH // 128):
                        v0 = vi * 128
                        pv = wpool.tile([128, HD], F32, tag="work",
                                        name="ps_v")
                        for kp in range(KP):
                            mm(pv, hsh_sb[:, 2 * kp:2 * kp + 2, v0:v0 + 128],
                               vh(kp), start=(kp == 0))
                        for kp in range(KP):
                            mm(pv, hsh_sb[:, 2 * kp:2 * kp + 2, v0:v0 + 128],
                               vl(kp))
                        psV.append(pv)
                    psQ = wpool.tile([128, CH], F32, tag="work", name="ps_q")
                    for kp in range(KP):
                        mm(psQ, qh(kp), mh(kp), start=(kp == 0))
                    for kp in range(KP):
                        mm(psQ, ql(kp), mh(kp))
                    # stage B: hsl-dependent third terms
                    for kp in range(KP):
                        mm(psK, kh(kp), ml(kp), stop=(kp == KP - 1))
                    rope(psK, kt_rot[:, t0:t0 + CH],
                         ck_sb[:, t0:t0 + CH], sk_sb[:, t0:t0 + CH])
                    for vi in range(CH // 128):
                        v0 = vi * 128
                        for kp in range(KP):
                            mm(psV[vi],
                               hsl_sb[:, 2 * kp:2 * kp + 2, v0:v0 + 128],
                               vh(kp), stop=(kp == KP - 1))
                        nc.scalar.copy(v_sb[:, t0 // 128 + vi, :], psV[vi])
                    for kp in range(KP):
                        mm(psQ, qh(kp), ml(kp), stop=(kp == KP - 1))
                    rope(psQ, qt_rot[:, 0, t0:t0 + CH],
                         cq_sb[:, t0:t0 + CH], sq_sb[:, t0:t0 + CH])
                    m_start = 1
                    active = None
                    created = 0
                # Q heads in quarter-batches; group h-1's units interleave
                # into head h's matmul stream (PE-heavy, Act-light)
                for m in range(m_start, HL):
                    ps = wpool.tile([128, CH], F32, tag="work", name="ps_q")
                    sh, sl = st(wqh_t[m]), st(wql_t[m])
                    mh, ml = mv(hsh_sb), mv(hsl_sb)
                    nb = 8 if qb else 4
                    for bi in range(nb):
                        k0 = bi * KP // nb
                        k1 = (bi + 1) * KP // nb
                        for kp in range(k0, k1):
                            nc.tensor.matmul(ps, sh(kp), mh(kp),
                                             start=(kp == 0), stop=False,
                                             perf_mode=DR)
                        for kp in range(k0, k1):
                            nc.tensor.matmul(ps, sl(kp), mh(kp),
                                             start=False, stop=False,
                                             perf_mode=DR)
                        for kp in range(k0, k1):
                            nc.tensor.matmul(ps, sh(kp), ml(kp), start=False,
                                             stop=(bi == nb - 1
                                                   and kp == k1 - 1),
                                             perf_mode=DR)
                        if active is not None:
                            drain(active, 1)
                    rope(ps, qt_rot[:, m, t0:t0 + CH],
                         cq_sb[:, t0:t0 + CH], sq_sb[:, t0:t0 + CH])
                    if created < m:
                        if active is not None:
                            drain(active)
                        active = group_units(b, created, qb)
                        created += 1
                # remaining groups (h=2,3) interleave with the O projection
                # of the previous chunk; chunk 0 has no O-proj to interleave.
                # Next-chunk hs DMAs (and wo, at c==0) pace into the stream.
                tail_gens = [active, group_units(b, HL - 1, qb)]
                active = None
                feed = []
                if c + 1 < NCHUNK:
                    hs_cur, feed = hs_dma_closures(c + 1)
                if c == 0:
                    feed.append(lambda: nc.sync.dma_start(woh_sb, woh_d.ap()))
                    feed.append(lambda: nc.sync.dma_start(wol_sb, wol_d.ap()))
                if c >= 1:
                    op = oproj_units(c - 1)
                    gi = 0
                    done_op = False
                    opn = 0
                    while not done_op:
                        if gi < len(tail_gens):
                            if drain(tail_gens[gi], 1):
                                gi += 1
                                continue
                        done_op = drain(op, 2 if qb else 4)
                        opn += 1
                        if feed and opn % 2 == 0:
                            feed.pop(0)()
                    for g in tail_gens[gi:]:
                        drain(g)
                else:
                    for f in feed:
                        f()
                    feed = []
                    for g in tail_gens:
                        drain(g)
                for f in feed:
                    f()
                if c == NCHUNK - 1 and pending[0] is not None:
                    # flush the last epilogue now: its DVE/Act ops run while
                    # the PE works through the final O projection below
                    epilogue(pending[0])
                    pending[0] = None
            drain(oproj_units(NCHUNK - 1))
    nc.compile()
    return nc


def _get_nc():
    if "nc" not in _CACHE:
        _CACHE["nc"] = _build()
    return _CACHE["nc"]


def _split8(x: np.ndarray):
    """Split f32 array into (hi, lo) e4m3 pair with hi + lo ~= x."""
    hi = x.astype(E4M3)
    lo = (x - hi.astype(np.float32)).astype(E4M3)
    return hi, lo


def _prep_inputs(inputs) -> list[dict]:
    bf16 = ml_dtypes.bfloat16
    hs = np.asarray(inputs["hidden_states"], dtype=np.float32).reshape(T, DIM)
    hsT = np.ascontiguousarray(hs.T)
    hsh, hsl = _split8(hsT)

    def swz_hs(x):  # [DIM, T] -> [128, NCHUNK, KT, CH] (SBUF layout)
        return np.ascontiguousarray(
            x.reshape(KT, 128, NCHUNK, CH).transpose(1, 2, 0, 3))

    hsh = swz_hs(hsh)
    hsl = swz_hs(hsl)

    fc = np.asarray(inputs["freqs_cos"], dtype=np.float32).reshape(T, HD // 2).T
    fs = np.asarray(inputs["freqs_sin"], dtype=np.float32).reshape(T, HD // 2).T
    cos2 = np.concatenate([fc, fc], axis=0)            # [128, T]
    sin2 = np.concatenate([-fs, fs], axis=0)           # signed half-rotation
    cos_qv = np.ascontiguousarray(cos2 * (SCALE / S_Q)).astype(bf16)
    sin_qv = np.ascontiguousarray(sin2 * (SCALE / S_Q)).astype(bf16)
    cos_kv = np.ascontiguousarray(cos2 * (1.0 / S_K)).astype(bf16)
    sin_kv = np.ascontiguousarray(sin2 * (1.0 / S_K)).astype(bf16)

    maskT = np.ascontiguousarray(
        np.asarray(inputs["attention_mask"], dtype=np.float32)[0, 0, :128, :128].T)

    perm = np.concatenate([np.arange(0, HD, 2), np.arange(1, HD, 2)])
    Wq = np.asarray(inputs["Wq"], dtype=np.float32)
    Wk = np.asarray(inputs["Wk"], dtype=np.float32)
    Wv = np.asarray(inputs["Wv"], dtype=np.float32)
    Wo = np.asarray(inputs["Wo"], dtype=np.float32)

    def swz_w(x, nh):  # [DIM, nh*HD] -> [128, nh, KT, HD]
        return np.ascontiguousarray(
            x.reshape(KT, 128, nh, HD).transpose(1, 2, 0, 3))

    in_maps = []
    for c in range(N_CORES):
        wq_c = np.concatenate(
            [Wq[:, (c * HL + h) * HD:(c * HL + h + 1) * HD][:, perm]
             for h in range(HL)], axis=1) * S_Q
        wk_c = Wk[:, c * HD:(c + 1) * HD][:, perm] * S_K
        wv_c = Wv[:, c * HD:(c + 1) * HD] * S_V
        wo_c = Wo[c * HL * HD:(c + 1) * HL * HD, :] * S_O
        wqh, wql = _split8(wq_c)
        wkh, wkl = _split8(wk_c)
        wvh, wvl = _split8(wv_c)
        woh, wol = _split8(wo_c)
        in_maps.append({
            "hsh": hsh, "hsl": hsl,
            "wqh": swz_w(wqh, HL), "wql": swz_w(wql, HL),
            "wkh": swz_w(wkh, 1).reshape(128, KT, HD),
            "wkl": swz_w(wkl, 1).reshape(128, KT, HD),
            "wvh": swz_w(wvh, 1).reshape(128, KT, HD),
            "wvl": swz_w(wvl, 1).reshape(128, KT, HD),
            "woh": np.ascontiguousarray(
                woh.reshape(HL, 128, DIM).transpose(1, 0, 2)),
            "wol": np.ascontiguousarray(
                wol.reshape(HL, 128, DIM).transpose(1, 0, 2)),
            "cos_q": cos_qv, "sin_q": sin_qv,
            "cos_k": cos_kv, "sin_k": sin_kv,
            "maskT": maskT,
        })
    return in_maps


def kernel(**inputs) -> np.ndarray:
    nc = _get_nc()
    in_maps = _prep_inputs(inputs)
    res = bass_utils.run_bass_kernel_spmd(nc, in_maps,
                                          core_ids=list(range(N_CORES)))
    acc = np.zeros((T, DIM), dtype=np.float32)
    for c in range(N_CORES):
        acc += np.asarray(res.results[c]["out"], dtype=np.float32)
    return (acc * (1.0 / (S_V * S_O))).reshape(B, S, DIM)
